# revision 3
# baseline (speedup 1.0000x reference)
"""MoE layer (top-2 of 8 experts) on 8 Trainium2 NeuronCores.

Strategy (d_ff-sharded / tensor-parallel over the expert hidden dim):
  * Host computes the (tiny) gating network: probs = softmax(x @ w_gate),
    top-2 experts + normalized gates per token, and sorts the N*K = 16384
    (token, expert) assignments by expert id.
  * Every core holds a 512-wide F-slice of ALL 8 experts' W1/W2 resident
    in SBUF (16 MB bf16) and processes ALL 16384 assignments, streaming
    the dispatched tokens through in expert-contiguous segments of <=256
    tokens that follow the EXACT expert boundaries -- zero token padding
    and perfect load balance across cores (the old expert-parallel layout
    padded every core to max expert load).
  * Each core computes the partial  o_c = gate * (relu(x @ W1[:, :, sl_c])
    @ W2[:, sl_c, :])  with the gate applied on-device during the
    PSUM->SBUF copy; the host sums the 8 partials (a pure reduction) and
    scatters rows back to token order.

Device kernel layout (per core, identical SPMD program on 8 cores):
  inputs  xd [128, 8*16384] bf16   dispatched tokens, segment-packed as
                                   (seg, ki, tok) columns so every segment
                                   load is one fully-contiguous DMA
          w1 [E, D, 512] bf16, w2 [E, 512, D] bf16   per-core F-slice
          g  [128, ntiles] f32     per-output-tile gate columns
  output  out [16384, D] f32       gated partial rows, dispatch order
  per segment s (expert e, L<=256 tokens):
    mm1: ph[f,tok] += w1[e,ki,f128].T @ x[ki]   (4 f-tiles x 8 k-tiles)
    relu -> h bf16 [f, tok]
    mm2: po[tok,d] += h[f128,tok].T @ w2[e,kf,d512]  (4 kf x 2 nd in PSUM)
    gate-scale on vector engine (PSUM -> SBUF), DMA rows out
  software pipeline: mm1(s+1) is emitted before mm2(s) so the PE never
  waits on the relu of the h-tile it is about to consume.
"""

import time

import numpy as np
import ml_dtypes

import concourse.bass as bass
import concourse.mybir as mybir
import concourse.tile as tile
from concourse import bacc
from concourse.bass_utils import run_bass_kernel_spmd

N, D, F, E, TOPK = 8192, 1024, 4096, 8, 2
P = 128
NCORES = 8
FS = F // NCORES          # 512  per-core f-slice
KD = D // P               # 8    k-tiles over d_model
KFS = FS // P             # 4    f-tiles in the slice
ND2 = D // 512            # 2    output n-tiles of 512
LMAX = 256                # max tokens per segment (psum: ph 2 banks x2)
NTOT = N * TOPK           # 16384 assignments

BF16 = mybir.dt.bfloat16
F32 = mybir.dt.float32

_program_cache: dict[tuple, "bass.Bass"] = {}
LAST_RESULTS = None    # BassKernelResults of the most recent run (for test.py)
LAST_SEGTAB = None     # segment table of the most recent run (for test.py)
TRACE = False          # test.py can flip this before calling kernel()


def _segtab_from_loads(loads):
    """Expert-contiguous segments of <=LMAX tokens, exact boundaries."""
    segs = []
    t0 = 0
    for e in range(E):
        le = int(loads[e])
        off = 0
        while off < le:
            seg_l = min(LMAX, le - off)
            segs.append((t0 + off, seg_l, e))
            off += seg_l
        t0 += le
    return tuple(segs)


def _tiles_of(segtab):
    """Output tiles (row0, ln) per segment, 128 tokens max each."""
    tiles = []
    for (t0, seg_l, _e) in segtab:
        for tm in range((seg_l + P - 1) // P):
            tiles.append((t0 + tm * P, min(P, seg_l - tm * P)))
    return tiles


def _build_program(segtab, bench_iters: int = 1) -> "bass.Bass":
    """Partial MoE FFN over this core's F-slice for all dispatched tokens.

    bench_iters > 1 wraps the compute in a hardware loop (same result, run
    repeatedly) so test harnesses can measure steady-state HW time from the
    wall-clock delta between two iteration counts."""
    S = len(segtab)
    tiles = _tiles_of(segtab)
    ntiles = len(tiles)
    ntot = sum(seg_l for _, seg_l, _ in segtab)

    nc = bacc.Bacc("TRN2", target_bir_lowering=False, debug=False,
                   num_devices=NCORES)
    xd = nc.dram_tensor("xd", [P, KD * ntot], BF16, kind="ExternalInput")
    w1 = nc.dram_tensor("w1", [E, D, FS], BF16, kind="ExternalInput")
    w2 = nc.dram_tensor("w2", [E, FS, D], BF16, kind="ExternalInput")
    g = nc.dram_tensor("g", [P, ntiles], F32, kind="ExternalInput")
    out = nc.dram_tensor("out", [ntot, D], F32, kind="ExternalOutput")

    w1_r = w1[:].rearrange("e (ki p) f -> e ki p f", p=P)
    w2_r = w2[:].rearrange("e (kf p) d -> e kf p d", p=P)

    # column offset of each segment in xd
    xoff = []
    acc = 0
    for (_t0, seg_l, _e) in segtab:
        xoff.append(acc)
        acc += KD * seg_l
    # first output-tile index of each segment
    tidx0 = []
    acc = 0
    for (_t0, seg_l, _e) in segtab:
        tidx0.append(acc)
        acc += (seg_l + P - 1) // P

    with tile.TileContext(nc) as tc:
        with (
            tc.tile_pool(name="wpool", bufs=1) as wpool,
            tc.tile_pool(name="xpool", bufs=4) as xpool,
            tc.tile_pool(name="hpool", bufs=3) as hpool,
            tc.tile_pool(name="opool", bufs=3) as opool,
            tc.tile_pool(name="ph_pool", bufs=2, space="PSUM") as ph_pool,
            tc.tile_pool(name="po_pool", bufs=2, space="PSUM") as po_pool,
        ):
            w1_sb = wpool.tile([P, E, KD, FS], BF16, name="w1_sb")
            w2_sb = wpool.tile([P, E, KFS, D], BF16, name="w2_sb")
            g_sb = wpool.tile([P, ntiles], F32, name="g_sb")
            nc.sync.dma_start(g_sb[:], g[:])
            for e in range(E):
                for k in range(KD):
                    nc.sync.dma_start(w1_sb[:, e, k, :], w1_r[e, k])
                for k in range(KFS):
                    nc.sync.dma_start(w2_sb[:, e, k, :], w2_r[e, k])

            def load_x(s):
                seg_l = segtab[s][1]
                xs = xpool.tile([P, KD * LMAX], BF16, name="xs", tag="xs")
                nc.sync.dma_start(xs[:, :KD * seg_l],
                                  xd[:, xoff[s]:xoff[s] + KD * seg_l])
                return xs

            def mm1(s, xs):
                _t0, seg_l, e = segtab[s]
                ph = ph_pool.tile([P, KFS * LMAX], F32, name="ph", tag="ph")
                h = hpool.tile([P, KFS * LMAX], BF16, name="h", tag="h")
                for ft in range(KFS):
                    for ki in range(KD):
                        nc.tensor.matmul(
                            ph[:, ft * LMAX:ft * LMAX + seg_l],
                            lhsT=w1_sb[:, e, ki, ft * P:(ft + 1) * P],
                            rhs=xs[:, ki * seg_l:(ki + 1) * seg_l],
                            start=(ki == 0),
                            stop=(ki == KD - 1),
                        )
                    nc.scalar.activation(
                        h[:, ft * LMAX:ft * LMAX + seg_l],
                        ph[:, ft * LMAX:ft * LMAX + seg_l],
                        mybir.ActivationFunctionType.Relu,
                    )
                return h

            def mm2(s, h):
                t0, seg_l, e = segtab[s]
                for tm in range((seg_l + P - 1) // P):
                    ln = min(P, seg_l - tm * P)
                    po = po_pool.tile([P, D], F32, name="po", tag="po")
                    for kf in range(KFS):
                        for nd in range(ND2):
                            nc.tensor.matmul(
                                po[:ln, nd * 512:(nd + 1) * 512],
                                lhsT=h[:, kf * LMAX + tm * P:
                                       kf * LMAX + tm * P + ln],
                                rhs=w2_sb[:, e, kf, nd * 512:(nd + 1) * 512],
                                start=(kf == 0),
                                stop=(kf == KFS - 1),
                            )
                    o = opool.tile([P, D], F32, name="o", tag="o")
                    j = tidx0[s] + tm
                    nc.vector.tensor_scalar_mul(
                        o[:ln, :], po[:ln, :], g_sb[:ln, j:j + 1])
                    nc.sync.dma_start(out[t0 + tm * P:t0 + tm * P + ln, :],
                                      o[:ln, :])

            def body():
                xs = {0: load_x(0)}
                if S > 1:
                    xs[1] = load_x(1)
                h_prev = mm1(0, xs[0])
                for s in range(S):
                    if s + 2 < S:
                        xs[s + 2] = load_x(s + 2)
                    h_next = mm1(s + 1, xs[s + 1]) if s + 1 < S else None
                    mm2(s, h_prev)
                    h_prev = h_next

            if bench_iters > 1:
                with tc.For_i(0, bench_iters, 1):
                    body()
            else:
                body()
    nc.compile()
    return nc


def _gate_and_dispatch(x, w_gate):
    """Replicates the reference gating exactly (fp32): softmax + top-2."""
    logits = x.astype(np.float32) @ w_gate.astype(np.float32)        # [N, E]
    m = logits.max(-1, keepdims=True)
    p = np.exp(logits - m)
    probs = p / p.sum(-1, keepdims=True)
    # jax.lax.top_k: descending, ties broken by lower index -> stable argsort
    tk_idx = np.argsort(-probs, axis=1, kind="stable")[:, :TOPK]
    tk_vals = np.take_along_axis(probs, tk_idx, axis=1)
    tk_gates = tk_vals / (tk_vals.sum(-1, keepdims=True) + 1e-9)
    return tk_idx, tk_gates


def kernel(x, w_gate, W1, W2):
    global LAST_RESULTS, LAST_SEGTAB
    x = np.asarray(x, dtype=np.float32)
    w_gate = np.asarray(w_gate, dtype=np.float32)
    W1 = np.asarray(W1, dtype=np.float32)
    W2 = np.asarray(W2, dtype=np.float32)
    n_tok = x.shape[0]
    ntot = n_tok * TOPK

    tk_idx, tk_gates = _gate_and_dispatch(x, w_gate)

    # dispatch: sort the (token, expert) assignments by expert id
    eid = tk_idx.reshape(-1).astype(np.int64)
    loads = np.bincount(eid, minlength=E)
    order = np.argsort(eid, kind="stable")
    tok_disp = (np.arange(ntot) // TOPK)[order]          # token of disp row
    g_disp = tk_gates.reshape(-1)[order].astype(np.float32)

    segtab = _segtab_from_loads(loads)
    tiles = _tiles_of(segtab)
    LAST_SEGTAB = segtab

    # xd: [128, KD*ntot] bf16, columns packed (seg, ki, tok) so each
    # segment's load is a single contiguous-per-partition DMA
    xb = np.ascontiguousarray(x[tok_disp].T).astype(ml_dtypes.bfloat16)
    xb = xb.reshape(KD, P, ntot)                          # (ki, p, n)
    xd = np.empty((P, KD * ntot), dtype=ml_dtypes.bfloat16)
    off = 0
    for (t0, seg_l, _e) in segtab:
        blk = xb[:, :, t0:t0 + seg_l].transpose(1, 0, 2).reshape(P, KD * seg_l)
        xd[:, off:off + KD * seg_l] = blk
        off += KD * seg_l

    # per-output-tile gate columns [128, ntiles]
    g_t = np.zeros((P, len(tiles)), dtype=np.float32)
    for j, (r0, ln) in enumerate(tiles):
        g_t[:ln, j] = g_disp[r0:r0 + ln]

    # per-core inputs: F-slice of all experts' weights
    in_maps = []
    for c in range(NCORES):
        sl = slice(c * FS, (c + 1) * FS)
        in_maps.append({
            "xd": xd,
            "w1": np.ascontiguousarray(W1[:, :, sl]).astype(ml_dtypes.bfloat16),
            "w2": np.ascontiguousarray(W2[:, sl, :]).astype(ml_dtypes.bfloat16),
            "g": g_t,
        })

    nc = _program_cache.get(segtab)
    if nc is None:
        nc = _build_program(segtab)
        _program_cache[segtab] = nc

    try:
        res = run_bass_kernel_spmd(nc, in_maps, core_ids=list(range(NCORES)),
                                   trace=TRACE)
    except Exception:
        # transient NRT/device hiccups (e.g. NRT_EXEC_UNIT_UNRECOVERABLE)
        # have been observed to clear after a short wait -- retry once
        time.sleep(20)
        res = run_bass_kernel_spmd(nc, in_maps, core_ids=list(range(NCORES)),
                                   trace=TRACE)
    LAST_RESULTS = res

    # combine: out rows are already gate-scaled partials; sum cores, then
    # sum each token's TOPK dispatch rows
    out_sum = np.array(res.results[0]["out"], dtype=np.float32)
    for c in range(1, NCORES):
        out_sum += np.asarray(res.results[c]["out"], dtype=np.float32)
    pos = np.empty(ntot, np.int64)
    pos[order] = np.arange(ntot)                         # flat idx -> disp row
    y = out_sum[pos[0::TOPK]] + out_sum[pos[1::TOPK]]
    return y.astype(np.float32)


# revision 20
# speedup vs baseline: 1.3469x; 1.3469x over previous
"""MoE layer (top-2 of 8 experts) on 8 Trainium2 NeuronCores.

Strategy (d_ff-sharded / tensor-parallel over the expert hidden dim):
  * Host computes the (tiny) gating network: probs = softmax(x @ w_gate),
    top-2 experts + normalized gates per token, and sorts the N*K = 16384
    (token, expert) assignments by expert id.
  * Every core holds a 512-wide F-slice of ALL 8 experts' W1/W2 resident
    in SBUF (16 MB bf16) and processes ALL 16384 assignments, streaming
    the dispatched tokens through in expert-contiguous segments of <=256
    tokens that follow the EXACT expert boundaries -- zero token padding
    and perfect load balance across cores (the old expert-parallel layout
    padded every core to max expert load).
  * Each core computes the partial  o_c = gate * (relu(x @ W1[:, :, sl_c])
    @ W2[:, sl_c, :])  with the gate applied on-device during the
    PSUM->SBUF copy; the host sums the 8 partials (a pure reduction) and
    scatters rows back to token order.

Device kernel layout (per core, identical SPMD program on 8 cores):
  inputs  xd [128, 8*16384] bf16   dispatched tokens, segment-packed as
                                   (seg, ki, tok) columns so every segment
                                   load is one fully-contiguous DMA
          w1 [E, D, 512] bf16, w2 [E, 512, D] bf16   per-core F-slice
          g  [128, ntiles] f32     per-output-tile gate columns
  output  out [16384, D] f32       gated partial rows, dispatch order
  per segment s (expert e, L<=256 tokens):
    mm1: ph[f,tok] += w1[e,ki,f128].T @ x[ki]   (4 f-tiles x 8 k-tiles)
    relu -> h bf16 [f, tok]
    mm2: po[tok,d] += h[f128,tok].T @ w2[e,kf,d512]  (4 kf x 2 nd in PSUM)
    gate-scale on vector engine (PSUM -> SBUF), DMA rows out
  software pipeline: mm1(s+1) is emitted before mm2(s) so the PE never
  waits on the relu of the h-tile it is about to consume.
"""

import time

import numpy as np
import ml_dtypes

import concourse.bass as bass
import concourse.mybir as mybir
import concourse.tile as tile
from concourse import bacc
from concourse.bass_utils import run_bass_kernel_spmd

N, D, F, E, TOPK = 8192, 1024, 4096, 8, 2
P = 128
NCORES = 8
FS = F // NCORES          # 512  per-core f-slice
KD = D // P               # 8    k-tiles over d_model
KFS = FS // P             # 4    f-tiles in the slice
ND2 = D // 512            # 2    output n-tiles of 512
LMAX = 256                # max tokens per segment (psum: ph 2 banks x2)
NTOT = N * TOPK           # 16384 assignments

BF16 = mybir.dt.bfloat16
F32 = mybir.dt.float32

# flags used for the real kernel() path and test.py's bench (must match):
# x loads batched 2 segments per DMA with 6 segments of prefetch, outputs
# on the sync queue in f32 (bf16 measured no faster; f32 is more accurate)
BEST_FLAGS: dict = {"_out_q": "sync", "_xbatch": 2, "_xahead": 6}

_program_cache: dict[tuple, "bass.Bass"] = {}
LAST_RESULTS = None    # BassKernelResults of the most recent run (for test.py)
LAST_SEGTAB = None     # segment table of the most recent run (for test.py)
TRACE = False          # test.py can flip this before calling kernel()


def _segtab_from_loads(loads):
    """Expert-contiguous segments of <=LMAX tokens, exact boundaries."""
    segs = []
    t0 = 0
    for e in range(E):
        le = int(loads[e])
        off = 0
        while off < le:
            seg_l = min(LMAX, le - off)
            segs.append((t0 + off, seg_l, e))
            off += seg_l
        t0 += le
    return tuple(segs)


def _tiles_of(segtab):
    """Output tiles (row0, ln) per segment, 128 tokens max each."""
    tiles = []
    for (t0, seg_l, _e) in segtab:
        for tm in range((seg_l + P - 1) // P):
            tiles.append((t0 + tm * P, min(P, seg_l - tm * P)))
    return tiles


def _build_program(segtab, bench_iters: int = 1,
                   _skip_out_dma: bool = False,
                   _static_x: bool = False,
                   _out_q: str = "sync",
                   _xbatch: int = 1,
                   _xahead: int = 2,
                   _out_bf16: bool = False) -> "bass.Bass":
    """Partial MoE FFN over this core's F-slice for all dispatched tokens.

    bench_iters > 1 wraps the compute in a hardware loop (same result, run
    repeatedly) so test harnesses can measure steady-state HW time from the
    wall-clock delta between two iteration counts."""
    S = len(segtab)
    tiles = _tiles_of(segtab)
    ntiles = len(tiles)
    ntot = sum(seg_l for _, seg_l, _ in segtab)

    nc = bacc.Bacc("TRN2", target_bir_lowering=False, debug=False,
                   num_devices=NCORES)
    xd = nc.dram_tensor("xd", [P, KD * ntot], BF16, kind="ExternalInput")
    w1 = nc.dram_tensor("w1", [E, D, FS], BF16, kind="ExternalInput")
    w2 = nc.dram_tensor("w2", [E, FS, D], BF16, kind="ExternalInput")
    g = nc.dram_tensor("g", [P, ntiles], F32, kind="ExternalInput")
    out_dt = BF16 if _out_bf16 else F32
    out = nc.dram_tensor("out", [ntot, D], out_dt, kind="ExternalOutput")

    w1_r = w1[:].rearrange("e (ki p) f -> e ki p f", p=P)
    w2_r = w2[:].rearrange("e (kf p) d -> e kf p d", p=P)

    # column offset of each segment in xd
    xoff = []
    acc = 0
    for (_t0, seg_l, _e) in segtab:
        xoff.append(acc)
        acc += KD * seg_l
    # first output-tile index of each segment
    tidx0 = []
    acc = 0
    for (_t0, seg_l, _e) in segtab:
        tidx0.append(acc)
        acc += (seg_l + P - 1) // P

    with tile.TileContext(nc) as tc:
        with (
            tc.tile_pool(name="wpool", bufs=1) as wpool,
            tc.tile_pool(name="xpool", bufs=_xahead // _xbatch + 2) as xpool,
            tc.tile_pool(name="hpool", bufs=3) as hpool,
            tc.tile_pool(name="opool", bufs=3) as opool,
            tc.tile_pool(name="ph_pool", bufs=2, space="PSUM") as ph_pool,
            tc.tile_pool(name="po_pool", bufs=2, space="PSUM") as po_pool,
        ):
            w1_sb = wpool.tile([P, E, KD, FS], BF16, name="w1_sb")
            w2_sb = wpool.tile([P, E, KFS, D], BF16, name="w2_sb")
            g_sb = wpool.tile([P, ntiles], F32, name="g_sb")
            nc.sync.dma_start(g_sb[:], g[:])
            for e in range(E):
                for k in range(KD):
                    nc.sync.dma_start(w1_sb[:, e, k, :], w1_r[e, k])
                for k in range(KFS):
                    nc.sync.dma_start(w2_sb[:, e, k, :], w2_r[e, k])

            x_static = None
            if _static_x:
                x_static = wpool.tile([P, KD * LMAX], BF16, name="x_static")
                nc.sync.dma_start(x_static[:], xd[:, :KD * LMAX])

            def load_x(s):
                """Loads segments [s, s+_xbatch) in one DMA; returns a dict
                seg -> (tile, col0) for each segment covered."""
                if _static_x:
                    return {si: (x_static, 0)
                            for si in range(s, min(s + _xbatch, S))}
                hi = min(s + _xbatch, S)
                ncols = xoff[hi - 1] + KD * segtab[hi - 1][1] - xoff[s]
                xs = xpool.tile([P, _xbatch * KD * LMAX], BF16,
                                name="xs", tag="xs")
                nc.sync.dma_start(xs[:, :ncols],
                                  xd[:, xoff[s]:xoff[s] + ncols])
                return {si: (xs, xoff[si] - xoff[s])
                        for si in range(s, hi)}

            def mm1(s, xs_entry):
                _t0, seg_l, e = segtab[s]
                xs, c0 = xs_entry
                ph = ph_pool.tile([P, KFS * LMAX], F32, name="ph", tag="ph")
                h = hpool.tile([P, KFS * LMAX], BF16, name="h", tag="h")
                for ft in range(KFS):
                    for ki in range(KD):
                        nc.tensor.matmul(
                            ph[:, ft * LMAX:ft * LMAX + seg_l],
                            lhsT=w1_sb[:, e, ki, ft * P:(ft + 1) * P],
                            rhs=xs[:, c0 + ki * seg_l:c0 + (ki + 1) * seg_l],
                            start=(ki == 0),
                            stop=(ki == KD - 1),
                        )
                    nc.scalar.activation(
                        h[:, ft * LMAX:ft * LMAX + seg_l],
                        ph[:, ft * LMAX:ft * LMAX + seg_l],
                        mybir.ActivationFunctionType.Relu,
                    )
                return h

            def mm2(s, h):
                t0, seg_l, e = segtab[s]
                for tm in range((seg_l + P - 1) // P):
                    ln = min(P, seg_l - tm * P)
                    po = po_pool.tile([P, D], F32, name="po", tag="po")
                    for kf in range(KFS):
                        for nd in range(ND2):
                            nc.tensor.matmul(
                                po[:ln, nd * 512:(nd + 1) * 512],
                                lhsT=h[:, kf * LMAX + tm * P:
                                       kf * LMAX + tm * P + ln],
                                rhs=w2_sb[:, e, kf, nd * 512:(nd + 1) * 512],
                                start=(kf == 0),
                                stop=(kf == KFS - 1),
                            )
                    o = opool.tile([P, D], out_dt, name="o", tag="o")
                    j = tidx0[s] + tm
                    nc.vector.tensor_scalar_mul(
                        o[:ln, :], po[:ln, :], g_sb[:ln, j:j + 1])
                    if not _skip_out_dma:
                        out_eng = {"sync": nc.sync, "scalar": nc.scalar,
                                   "gpsimd": nc.gpsimd}[_out_q]
                        out_eng.dma_start(
                            out[t0 + tm * P:t0 + tm * P + ln, :], o[:ln, :])

            def body():
                xs = {}
                for s0 in range(0, min(_xahead + _xbatch, S), _xbatch):
                    xs.update(load_x(s0))
                h_prev = mm1(0, xs[0])
                for s in range(S):
                    nxt = s + _xahead + _xbatch
                    if nxt < S and nxt % _xbatch == 0:
                        xs.update(load_x(nxt))
                    h_next = mm1(s + 1, xs[s + 1]) if s + 1 < S else None
                    mm2(s, h_prev)
                    h_prev = h_next

            if bench_iters > 1:
                with tc.For_i(0, bench_iters, 1):
                    body()
            else:
                body()
            if _skip_out_dma:
                nc.sync.dma_start(out[0:P, 0:ntiles], g_sb[:])
    nc.compile()
    return nc


def _gate_and_dispatch(x, w_gate):
    """Replicates the reference gating exactly (fp32): softmax + top-2."""
    logits = x.astype(np.float32) @ w_gate.astype(np.float32)        # [N, E]
    m = logits.max(-1, keepdims=True)
    p = np.exp(logits - m)
    probs = p / p.sum(-1, keepdims=True)
    # jax.lax.top_k: descending, ties broken by lower index -> stable argsort
    tk_idx = np.argsort(-probs, axis=1, kind="stable")[:, :TOPK]
    tk_vals = np.take_along_axis(probs, tk_idx, axis=1)
    tk_gates = tk_vals / (tk_vals.sum(-1, keepdims=True) + 1e-9)
    return tk_idx, tk_gates


def kernel(x, w_gate, W1, W2):
    global LAST_RESULTS, LAST_SEGTAB
    x = np.asarray(x, dtype=np.float32)
    w_gate = np.asarray(w_gate, dtype=np.float32)
    W1 = np.asarray(W1, dtype=np.float32)
    W2 = np.asarray(W2, dtype=np.float32)
    n_tok = x.shape[0]
    ntot = n_tok * TOPK

    tk_idx, tk_gates = _gate_and_dispatch(x, w_gate)

    # dispatch: sort the (token, expert) assignments by expert id
    eid = tk_idx.reshape(-1).astype(np.int64)
    loads = np.bincount(eid, minlength=E)
    order = np.argsort(eid, kind="stable")
    tok_disp = (np.arange(ntot) // TOPK)[order]          # token of disp row
    g_disp = tk_gates.reshape(-1)[order].astype(np.float32)

    segtab = _segtab_from_loads(loads)
    tiles = _tiles_of(segtab)
    LAST_SEGTAB = segtab

    # xd: [128, KD*ntot] bf16, columns packed (seg, ki, tok) so each
    # segment's load is a single contiguous-per-partition DMA
    xb = np.ascontiguousarray(x[tok_disp].T).astype(ml_dtypes.bfloat16)
    xb = xb.reshape(KD, P, ntot)                          # (ki, p, n)
    xd = np.empty((P, KD * ntot), dtype=ml_dtypes.bfloat16)
    off = 0
    for (t0, seg_l, _e) in segtab:
        blk = xb[:, :, t0:t0 + seg_l].transpose(1, 0, 2).reshape(P, KD * seg_l)
        xd[:, off:off + KD * seg_l] = blk
        off += KD * seg_l

    # per-output-tile gate columns [128, ntiles]
    g_t = np.zeros((P, len(tiles)), dtype=np.float32)
    for j, (r0, ln) in enumerate(tiles):
        g_t[:ln, j] = g_disp[r0:r0 + ln]

    # per-core inputs: F-slice of all experts' weights
    in_maps = []
    for c in range(NCORES):
        sl = slice(c * FS, (c + 1) * FS)
        in_maps.append({
            "xd": xd,
            "w1": np.ascontiguousarray(W1[:, :, sl]).astype(ml_dtypes.bfloat16),
            "w2": np.ascontiguousarray(W2[:, sl, :]).astype(ml_dtypes.bfloat16),
            "g": g_t,
        })

    nc = _program_cache.get(segtab)
    if nc is None:
        nc = _build_program(segtab, **BEST_FLAGS)
        _program_cache[segtab] = nc

    try:
        res = run_bass_kernel_spmd(nc, in_maps, core_ids=list(range(NCORES)),
                                   trace=TRACE)
    except Exception:
        # transient NRT/device hiccups (e.g. NRT_EXEC_UNIT_UNRECOVERABLE)
        # have been observed to clear after a short wait -- retry once
        time.sleep(20)
        res = run_bass_kernel_spmd(nc, in_maps, core_ids=list(range(NCORES)),
                                   trace=TRACE)
    LAST_RESULTS = res

    # combine: out rows are already gate-scaled partials; sum cores, then
    # sum each token's TOPK dispatch rows
    out_sum = np.array(res.results[0]["out"], dtype=np.float32)
    for c in range(1, NCORES):
        out_sum += np.asarray(res.results[c]["out"], dtype=np.float32)
    pos = np.empty(ntot, np.int64)
    pos[order] = np.arange(ntot)                         # flat idx -> disp row
    y = out_sum[pos[0::TOPK]] + out_sum[pos[1::TOPK]]
    return y.astype(np.float32)
